# revision 1
# baseline (speedup 1.0000x reference)
"""CPC loss kernel for Trainium2, data-parallel over 8 NeuronCores.

Math
----
Reference (per row x of shape [C], target t, y = x[t], C = 128):
  ce   = logsumexp(x) - y
  bdc  = -(sum_{j != t} log_sigmoid(y - x_j)) / (C-1)
  bec  = -(0.5 * sum_{j,k in rest} log_sigmoid(x_j - x_k + EPS)) / ((C-1)(C-2))

With log_sigmoid(d) = -softplus(-d), extending the rest-pair sums to the full
C x C square plus O(C) corrections (EPS = 1e-10 is invisible in fp32):

  P1 = sum_j sp(x_j - y),  P2 = sum_j sp(y - x_j)     # full C each
  SP = sum_{j,k in C x C} sp(x_j - x_k)               # diagonal included
  row_loss = (mx + ln(sum e^{x-mx}) - y)
           + (P1 - log2)/(C-1) + 0.5*(SP - P1 - P2 + log2)/((C-1)(C-2))

The ACT tables in this toolchain have no softplus, so sp comes from
Exp + Ln(bias=1) (one table set: natural_log_exp_and_others), and the pair
count is halved with sp(d) + sp(-d) = 2*ln(1+e^d) - d:

  SP = 2*LNS - LC,   LNS = sum_{j<k} ln(1+e^{d_jk}) + npad*ln2  (measured,
       with npad = 64 zero pad columns; diagonal C*ln2 = 2*npad*ln2 cancels)
  LC = sum_i x_i * (C-1-2i)          # linear, on VectorE

Kernel structure (per core, 2048 rows as 16 batches of 128):
  - TensorE: D[r, f] = x_r,j(f) - x_r,k(f) over the 8128 j<k pairs (+64 pad)
    via lhsT = X^T (bf16) times constant W[kappa, f] = delta(kappa, j(f)) -
    delta(kappa, k(f)), into [128, 2048] PSUM chunks.
  - ScalarE: Exp then Ln(bias=1, accum_out) in-place on each PSUM chunk;
    P1/P2 via the per-partition bias port (bias = -y / +y); CE sumexp.
  - VectorE: max, target gather (iota == t mask), LC, final combine.
Per-row losses are DMA'd out; the host sums across rows and cores.
"""

import functools

import numpy as np
import ml_dtypes

import concourse.bass as bass
import concourse.tile as tile
import concourse.hw_specs as hw_specs
from concourse import bacc, mybir
from concourse.bass_utils import run_bass_kernel_spmd

# The act-table chooser greedily picks the first set containing each
# function, so an exp/ln-alternating kernel loads exp_and_others and
# natural_log in alternation (~2.7us per load, ~96 loads here). Blank the
# single-function sets (keeping dict order, so act_func_set_id indices into
# act_info.json stay valid) so both exp and ln resolve to
# natural_log_exp_and_others and a single load suffices.
_orig_get_activation_tables = hw_specs.get_activation_tables


@functools.cache
def _patched_activation_tables(module_arch: str):
    d = dict(_orig_get_activation_tables(module_arch))
    for name in ("exp_and_others", "natural_log", "exp_and_friends"):
        if name in d:
            d[name] = set()
    return d


hw_specs.get_activation_tables = _patched_activation_tables
bacc.get_activation_tables = _patched_activation_tables

N, C = 16384, 128
NCORES = 8
ROWS = N // NCORES            # rows per core
P = 128                       # partitions / rows per batch
NB = ROWS // P                # batches per core
NPAIR = (C * (C - 1)) // 2    # 8128
NPAD = 64
NF = NPAIR + NPAD             # 8192 pair columns
CHUNK = 2048                  # free elems per ACT instruction (4 PSUM banks)
NCHUNK = NF // CHUNK          # 4 chunks per batch
MM_N = 512                    # moving free dim per matmul (1 PSUM bank)

F32 = mybir.dt.float32
BF16 = mybir.dt.bfloat16
AF = mybir.ActivationFunctionType
ALU = mybir.AluOpType

LOG2 = float(np.log(2.0))
C_SP = 0.5 / ((C - 1) * (C - 2))          # "c"
# row_loss = ce - (sum_{j!=t} ls(y-x_j))/(C-1) - 0.5*T/((C-1)(C-2)) with
# ce computed as ln(sum_j e^{x_j - y}) (y-shifted logsumexp: x-y <= ~10 for
# randn inputs so no overflow, and the same e^{x-y} feeds P1), P2 recovered
# from P2 = P1 - S + C*Y (S = sum_j x_j, the sp(d)+sp(-d) identity):
# row_loss = LSE + K_Y*Y + K_P1*P1 + K_S*S + 2c*LNS - c*LC + C_CONST
K_Y = -C * C_SP
K_P1 = 1.0 / (C - 1) - 2.0 * C_SP
K_S = C_SP
C_CONST = -LOG2 / (C - 1) + 0.5 * LOG2 / ((C - 1) * (C - 2))

# Number of the 4 pair-chunks per batch whose ln-sum goes through the
# VectorE group-product path (sum ln(1+u) = sum over groups of ln prod(1+u),
# groups of 8 so fp32 can't overflow) instead of a full-width ACT Ln pass.
# Balances the ScalarE (sole exp/ln engine) against the otherwise idle DVE.
N_PROD_DEFAULT = 4

_cache: dict = {}


def _build_program(repeat: int = 1, n_prod: int = N_PROD_DEFAULT) -> bass.Bass:
    # Bacc (not raw Bass): its compile() runs generate_event_semaphores,
    # which splits multi-sem waits (the ACT ISA has a single wait slot).
    nc = bacc.Bacc("TRN2")

    x_d = nc.declare_dram_parameter("x", [ROWS, C], F32, isOutput=False)
    xt_d = nc.declare_dram_parameter("xt", [C, ROWS], BF16, isOutput=False)
    w_d = nc.declare_dram_parameter("w", [C, NF], BF16, isOutput=False)
    io_d = nc.declare_dram_parameter("io", [P, C], F32, isOutput=False)
    cf_d = nc.declare_dram_parameter("cf", [P, C], F32, isOutput=False)
    tf_d = nc.declare_dram_parameter("tf", [ROWS], F32, isOutput=False)
    out_d = nc.declare_dram_parameter("out", [ROWS], F32, isOutput=True)

    with tile.TileContext(nc) as tc:
        with (
            tc.tile_pool(name="const", bufs=1) as const_pool,
            tc.tile_pool(name="work", bufs=3) as work,
            tc.tile_pool(name="acc", bufs=1) as acc_pool,
            tc.tile_pool(name="psum", bufs=2, space="PSUM") as psum_pool,
        ):
            # load order: small tensors and xt first so batch-0 work can
            # start while the 2MB W streams in (in chunk-sized pieces)
            io_sb = const_pool.tile([P, C], F32)
            nc.sync.dma_start(out=io_sb, in_=io_d[:])
            cf_sb = const_pool.tile([P, C], F32)
            nc.sync.dma_start(out=cf_sb, in_=cf_d[:])
            t_sb = const_pool.tile([P, NB], F32)
            nc.sync.dma_start(out=t_sb, in_=tf_d.rearrange("(b p) -> p b", p=P))
            xt_sb = const_pool.tile([C, ROWS], BF16)
            nc.sync.dma_start(out=xt_sb, in_=xt_d[:])
            x_sb = const_pool.tile([P, NB, C], F32)
            nc.sync.dma_start(out=x_sb, in_=x_d.rearrange("(b p) c -> p b c", p=P))
            w_sb = const_pool.tile([C, NF], BF16)
            for ch in range(NCHUNK):
                nc.sync.dma_start(
                    out=w_sb[:, ch * CHUNK : (ch + 1) * CHUNK],
                    in_=w_d[:, ch * CHUNK : (ch + 1) * CHUNK],
                )

            LNS = acc_pool.tile([P, NB], F32)
            LC = acc_pool.tile([P, NB], F32)
            P1 = acc_pool.tile([P, NB], F32)
            SU = acc_pool.tile([P, NB], F32)
            SE = acc_pool.tile([P, NB], F32)
            Y = acc_pool.tile([P, NB], F32)
            NY = acc_pool.tile([P, NB], F32)

            for _rep in range(repeat):
              for b in range(NB):
                xb = x_sb[:, b, :]
                yb = Y[:, b : b + 1]
                nyb = NY[:, b : b + 1]

                # y = x[r, t_r] via (iota == t) mask then masked row-sum
                # (tensor_tensor_reduce is a custom DVE op that dies at
                # runtime here, so use plain mul + reduce)
                mask = work.tile([P, C], F32, tag="mask")
                nc.vector.tensor_scalar(
                    mask, io_sb, t_sb[:, b : b + 1], None, op0=ALU.is_equal
                )
                nc.vector.tensor_mul(mask, mask, xb)
                nc.vector.tensor_reduce(
                    yb, mask, axis=mybir.AxisListType.X, op=ALU.add
                )
                nc.vector.tensor_scalar_mul(nyb, yb, -1.0)

                # LC = sum_i x_i * (C-1-2i)
                prod = work.tile([P, C], F32, tag="prod")
                nc.vector.tensor_mul(prod, xb, cf_sb)
                nc.vector.tensor_reduce(
                    LC[:, b : b + 1], prod, axis=mybir.AxisListType.X, op=ALU.add
                )

                # u1 = e^{x - y} feeds both P1 (ln(1+u) via the product
                # path) and the y-shifted CE logsumexp (sum u -> ln at end)
                scr1 = work.tile([P, C], F32, tag="scr1")
                nc.scalar.activation(scr1, xb, AF.Exp, bias=nyb, scale=1.0)
                nc.vector.tensor_reduce(
                    SE[:, b : b + 1], scr1, axis=mybir.AxisListType.X, op=ALU.add
                )
                p1u = work.tile([P, C], BF16, tag="p1u")
                nc.vector.tensor_scalar_add(p1u, scr1, 1.0)
                nc.vector.tensor_mul(p1u[:, :64], p1u[:, :64], p1u[:, 64:128])
                nc.vector.tensor_mul(p1u[:, :32], p1u[:, :32], p1u[:, 32:64])
                nc.vector.tensor_mul(p1u[:, :16], p1u[:, :16], p1u[:, 16:32])
                p1scr = work.tile([P, 16], F32, tag="p1scr")
                nc.scalar.activation(
                    p1scr, p1u[:, :16], AF.Ln, bias=0.0, scale=1.0,
                    accum_out=P1[:, b : b + 1],
                )
                # S = sum_j x_j
                nc.vector.tensor_reduce(
                    SU[:, b : b + 1], xb, axis=mybir.AxisListType.X, op=ALU.add
                )

                # LNS over the 8192 pair columns
                lnacc = (
                    work.tile([P, NCHUNK], F32, tag="lnacc")
                    if n_prod < NCHUNK
                    else None
                )
                G = CHUNK // 8  # group-products per chunk
                lnin = work.tile([P, NCHUNK * G], BF16, tag="lnin")
                lhsT = xt_sb[:, b * P : (b + 1) * P]
                for ch in range(NCHUNK):
                    pt = psum_pool.tile([P, CHUNK], F32, tag="pair")
                    for m in range(CHUNK // MM_N):
                        f0 = ch * CHUNK + m * MM_N
                        nc.tensor.matmul(
                            pt[:, m * MM_N : (m + 1) * MM_N],
                            lhsT,
                            w_sb[:, f0 : f0 + MM_N],
                        )
                    if ch < n_prod:
                        # DVE product path: u -> 1+u -> products of 8 ->
                        # one short Ln per batch. Frees ScalarE, which is the
                        # bottleneck. bf16 scratch: the +1 runs in DVE 4x
                        # packed mode and the multiply tree in 2x (vs 2x/1x
                        # for f32); the rounding noise is random-sign and
                        # vanishes in the 16K-row mean.
                        eu = work.tile([P, CHUNK], BF16, tag="eu")
                        nc.scalar.activation(eu, pt, AF.Exp, bias=0.0, scale=1.0)
                        nc.vector.tensor_scalar_add(eu, eu, 1.0)
                        h = CHUNK // 2
                        nc.vector.tensor_mul(eu[:, :h], eu[:, :h], eu[:, h:])
                        nc.vector.tensor_mul(
                            eu[:, : h // 2], eu[:, : h // 2], eu[:, h // 2 : h]
                        )
                        nc.vector.tensor_mul(
                            lnin[:, ch * G : (ch + 1) * G],
                            eu[:, : h // 4],
                            eu[:, h // 4 : h // 2],
                        )
                    else:
                        nc.scalar.activation(pt, pt, AF.Exp, bias=0.0, scale=1.0)
                        nc.scalar.activation(
                            pt, pt, AF.Ln, bias=1.0, scale=1.0,
                            accum_out=lnacc[:, ch : ch + 1],
                        )
                # one Ln + accum over all product-chunk groups at once
                if n_prod > 0:
                    lnscr = work.tile([P, n_prod * G], F32, tag="lnscr")
                    nc.scalar.activation(
                        lnscr, lnin[:, : n_prod * G], AF.Ln, bias=0.0, scale=1.0,
                        accum_out=LNS[:, b : b + 1],
                    )
                else:
                    nc.vector.memset(LNS[:, b : b + 1], 0.0)
                if n_prod < NCHUNK:
                    nc.vector.tensor_reduce(
                        lnacc[:, 0:1], lnacc[:, n_prod:NCHUNK],
                        axis=mybir.AxisListType.X, op=ALU.add,
                    )
                    nc.vector.tensor_add(
                        LNS[:, b : b + 1], LNS[:, b : b + 1], lnacc[:, 0:1]
                    )

            LSE = acc_pool.tile([P, NB], F32)
            nc.scalar.activation(LSE, SE, AF.Ln)

            # row_loss = LSE + K_Y*Y + K_P1*P1 + K_S*S
            #          + (2*C_SP)*LNS - C_SP*LC + C_CONST
            L = acc_pool.tile([P, NB], F32)
            T1 = acc_pool.tile([P, NB], F32)
            nc.vector.tensor_scalar_mul(T1, Y, K_Y)
            nc.vector.tensor_add(L, LSE, T1)
            nc.vector.tensor_scalar_mul(T1, P1, K_P1)
            nc.vector.tensor_add(L, L, T1)
            nc.vector.tensor_scalar_mul(T1, SU, K_S)
            nc.vector.tensor_add(L, L, T1)
            nc.vector.tensor_scalar_mul(T1, LNS, 2.0 * C_SP)
            nc.vector.tensor_add(L, L, T1)
            nc.vector.tensor_scalar_mul(T1, LC, -C_SP)
            nc.vector.tensor_add(L, L, T1)
            nc.vector.tensor_scalar_add(L, L, C_CONST)

            nc.sync.dma_start(out=out_d.rearrange("(b p) -> p b", p=P), in_=L)

    nc.compile()
    return nc


def _host_constants():
    if "w" not in _cache:
        ju, ku = np.triu_indices(C, 1)
        w = np.zeros((C, NF), np.float32)
        f = np.arange(NPAIR)
        w[ju, f] = 1.0
        w[ku, f] = -1.0
        _cache["w"] = w.astype(ml_dtypes.bfloat16)
        _cache["io"] = np.broadcast_to(
            np.arange(C, dtype=np.float32), (P, C)
        ).copy()
        _cache["cf"] = np.broadcast_to(
            (C - 1 - 2 * np.arange(C)).astype(np.float32), (P, C)
        ).copy()
    return _cache["w"], _cache["io"], _cache["cf"]


def kernel(inputs: np.ndarray, targets: np.ndarray) -> np.ndarray:
    x = np.ascontiguousarray(np.asarray(inputs, dtype=np.float32))
    t = np.asarray(targets)
    assert x.shape == (N, C) and t.shape == (N,)

    if "nc" not in _cache:
        _cache["nc"] = _build_program()
    nc = _cache["nc"]
    w, io, cf = _host_constants()

    xt = np.ascontiguousarray(x.T).astype(ml_dtypes.bfloat16)
    tf = t.astype(np.float32)

    in_maps = []
    for c in range(NCORES):
        r0, r1 = c * ROWS, (c + 1) * ROWS
        in_maps.append(
            {
                "x": np.ascontiguousarray(x[r0:r1]),
                "xt": np.ascontiguousarray(xt[:, r0:r1]),
                "w": w,
                "io": io,
                "cf": cf,
                "tf": np.ascontiguousarray(tf[r0:r1]),
            }
        )

    res = run_bass_kernel_spmd(nc, in_maps, list(range(NCORES)))
    total = 0.0
    for c in range(NCORES):
        total += np.sum(res.results[c]["out"].astype(np.float64))
    return np.float32(total / N)



# revision 2
# speedup vs baseline: 1.1817x; 1.1817x over previous
"""CPC loss kernel for Trainium2, data-parallel over 8 NeuronCores.

Math (per row x of shape [C], target t, y = x[t], C = 128, sp(d) = ln(1+e^d)):
  ce   = ln(sum_j e^{x_j}) - y
  bdc  = (P1 - ln2)/(C-1),                P1 = sum_j sp(x_j - y)
  bec  = 0.5*(SP - 2*P1 + S - C*y + ln2)/((C-1)(C-2)),
         SP = sum_{j,k in CxC} sp(x_j - x_k),  S = sum_j x_j

Pair enumeration is CIRCULANT: ordered pairs (j, (j+delta)%C), delta=1..127.
Pairing delta with C-delta and using sp(d)+sp(-d) = 2*ln(1+e^d) - d (the d's
telescope to zero over a full cyclic shift):

  SP = C*ln2 + 2*sum_{delta=1..63} sum_j sp(d_{j,delta})
            + sum_j sp(d_{j,64}),      d_{j,delta} = x_j - x_{(j+delta)%C}

No linear correction terms at all.  Only the SUM over rows is needed (the
output is a scalar), so all per-row quantities accumulate linearly: group
products are buffered across all 16 row-batches and a handful of big
Ln(accum_out) instructions run once at the end.

Work split per 128-row batch (8192 pair columns = 64 delta-blocks of 128):
  - delta 1..48 (3 chunks of 2048): TensorE matmul W gives e = x_{j+d} - x_j
    in PSUM; ScalarE computes sigma(e) = 1/(1+e^{d}) [one Sigmoid pass, no
    "+1" needed]; DVE product-trees groups of 8; ln sigma = -sp(d).
  - delta 49..64 (1 chunk): "rank-1" path with NO ScalarE work per pair:
    u = e^{x_j}*e^{-x_{j+d}} from precomputed a = e^x, b2 = [e^-x, e^-x]
    via a broadcast AP times a sliding-window AP (one DVE mul), then
    w = (1+u)*e^-LAM in one fused 4x tensor_scalar, product-trees of 16.
    delta=64 (weight 1, not 2) gets its own sub-tree and accumulator.
  - P1 via the same rank-1 trick with b_t = e^{-y} as a per-partition scalar.
  - CE: ln(sum_j a_j) - y; sum_j a_j is a DVE reduce of a.
Tables: exp+ln live in one set (natural_log_exp_and_others, via the chooser
patch below), Sigmoid in sigmoid_and_others -> exactly 3 table loads
(exp phase -> sigmoid phase -> ln tail).
Output: per-partition partial sums [P,1]; host sums 128*8 values.
"""

import functools

import numpy as np
import ml_dtypes

import concourse.bass as bass
import concourse.tile as tile
import concourse.hw_specs as hw_specs
from concourse import bacc, mybir
from concourse.ap import AP
from concourse.bass_utils import run_bass_kernel_spmd

# The act-table chooser greedily picks the first set containing each
# function; blank the single-function sets (keeping dict order) so exp and
# ln both resolve to natural_log_exp_and_others and the only table swaps are
# exp-set -> sigmoid_and_others -> exp-set.
_orig_get_activation_tables = hw_specs.get_activation_tables


@functools.cache
def _patched_activation_tables(module_arch: str):
    d = dict(_orig_get_activation_tables(module_arch))
    for name in ("exp_and_others", "natural_log", "exp_and_friends"):
        if name in d:
            d[name] = set()
    return d


hw_specs.get_activation_tables = _patched_activation_tables
bacc.get_activation_tables = _patched_activation_tables

N, C = 16384, 128
NCORES = 8
ROWS = N // NCORES            # rows per core
P = 128                       # partitions / rows per batch
NB = ROWS // P                # batches per core
CHUNK = 2048                  # pair columns per chunk (16 delta-blocks)
NCH_ACT = 3                   # sigma-path chunks (delta 1..48)
WCOLS = NCH_ACT * CHUNK       # matmul weight columns
MM_N = 512                    # moving free dim per matmul (1 PSUM bank)

F32 = mybir.dt.float32
BF16 = mybir.dt.bfloat16
AF = mybir.ActivationFunctionType
ALU = mybir.AluOpType

LAM = 4.4                     # rank-1 rescale so groups of 16 fit fp32/bf16
ELAM = float(np.exp(-LAM))
LOG2 = float(np.log(2.0))
M2 = (C - 1) * (C - 2)

# Per-partition group counts (16 batches) and the LAM corrections their
# ln-sums carry (each rank-1 group of 16 factors carries e^{-16*LAM}).
NG_R1 = NB * 120              # delta 49..63 groups of 16
NG_R64 = NB * 8               # delta 64 groups of 16
K1 = NG_R1 * 16 * LAM
K64 = NG_R64 * 16 * LAM

# Sum over a partition's rows of the loss:
#   L = ACC_CE + K_P1*ACC_P1 + K_R1*ACC_R1 + K_S*ACC_S + K_R64*ACC_R64
#     + K_SX*SX + K_Y*SY + CONST_L
# where ACC_S = sum ln(sigma groups) = -sum sp over delta<=48,
# ACC_R1/_R64 = sum ln(rank-1 groups) (pre-LAM-correction), ACC_P1 = P1 sum,
# ACC_CE = sum ln(sum_j e^x), SX = sum of all x, SY = sum of targets' logits.
CONST_T = 2.0 * K1 + K64 + NB * C * LOG2 + NB * LOG2
K_CE = 1.0
K_P1 = 1.0 / (C - 1) - 1.0 / M2
K_R1 = 1.0 / M2
K_S = -1.0 / M2
K_R64 = 0.5 / M2
K_SX = 0.5 / M2
K_Y = -1.0 - 0.5 * C / M2
CONST_L = -NB * LOG2 / (C - 1) + 0.5 * CONST_T / M2

_cache: dict = {}


def _build_program() -> bass.Bass:
    nc = bacc.Bacc("TRN2")

    x_d = nc.declare_dram_parameter("x", [ROWS, C], F32, isOutput=False)
    xt_d = nc.declare_dram_parameter("xt", [C, ROWS], BF16, isOutput=False)
    w_d = nc.declare_dram_parameter("w", [C, WCOLS], BF16, isOutput=False)
    io_d = nc.declare_dram_parameter("io", [P, C], BF16, isOutput=False)
    cf_d = nc.declare_dram_parameter("cf", [P, 8], F32, isOutput=False)
    tf_d = nc.declare_dram_parameter("tf", [ROWS], F32, isOutput=False)
    out_d = nc.declare_dram_parameter("out", [P, 1], F32, isOutput=True)

    with tile.TileContext(nc) as tc:
        with (
            tc.tile_pool(name="const", bufs=1) as const_pool,
            tc.tile_pool(name="work", bufs=3) as work,
            tc.tile_pool(name="acc", bufs=1) as acc_pool,
            tc.tile_pool(name="psum", bufs=2, space="PSUM") as psum_pool,
        ):
            # x first: the exp phase depends only on it
            x_sb = const_pool.tile([P, NB, C], F32)
            nc.sync.dma_start(out=x_sb, in_=x_d.rearrange("(b p) c -> p b c", p=P))
            io_sb = const_pool.tile([P, C], BF16)
            nc.sync.dma_start(out=io_sb, in_=io_d[:])
            cf_sb = const_pool.tile([P, 8], F32)
            nc.sync.dma_start(out=cf_sb, in_=cf_d[:])
            t_sb = const_pool.tile([P, NB], F32)
            nc.sync.dma_start(out=t_sb, in_=tf_d.rearrange("(b p) -> p b", p=P))
            xt_sb = const_pool.tile([C, ROWS], BF16)
            nc.sync.dma_start(out=xt_sb, in_=xt_d[:])
            w_sb = const_pool.tile([C, WCOLS], BF16)
            for ch in range(NCH_ACT):
                nc.sync.dma_start(
                    out=w_sb[:, ch * CHUNK : (ch + 1) * CHUNK],
                    in_=w_d[:, ch * CHUNK : (ch + 1) * CHUNK],
                )

            a_sb = acc_pool.tile([P, NB, C], BF16)       # e^x
            b2 = acc_pool.tile([P, NB, 2 * C], BF16)     # [e^-x, e^-x]
            bt = acc_pool.tile([P, NB], F32)             # e^-y per row
            Y = acc_pool.tile([P, NB], F32)              # y per row
            SE = acc_pool.tile([P, NB], F32)             # sum_j e^x per row
            XS = acc_pool.tile([P, NB], F32)             # sum_j x per row
            gs_all = acc_pool.tile([P, NB, NCH_ACT * 256], BF16)
            r1_all = acc_pool.tile([P, NB, 120], BF16)
            r64_all = acc_pool.tile([P, NB, 8], BF16)
            p1_all = acc_pool.tile([P, NB, 16], BF16)
            ACCS = acc_pool.tile([P, 8], F32)            # CE,P1,R1,S,R64,SX,Y,1
            mask_all = acc_pool.tile([P, NB, C], BF16)
            Lfin = acc_pool.tile([P, 1], F32)

            # ---- phase E: exps + gathers (exp table on ACT; DVE prework)
            nc.scalar.activation(a_sb[:, :, :], x_sb[:, :, :], AF.Exp)
            nc.scalar.activation(
                b2[:, :, 0:C], x_sb[:, :, :], AF.Exp, bias=0.0, scale=-1.0
            )
            nc.vector.tensor_copy(b2[:, :, C : 2 * C], b2[:, :, 0:C])

            for b in range(NB):
                nc.vector.tensor_scalar(
                    mask_all[:, b, :], io_sb, t_sb[:, b : b + 1], None,
                    op0=ALU.is_equal,
                )
            xm = acc_pool.tile([P, NB, C], F32)
            nc.vector.tensor_mul(xm, x_sb, mask_all)
            nc.vector.tensor_reduce(Y, xm, axis=mybir.AxisListType.X, op=ALU.add)
            bm = acc_pool.tile([P, NB, C], BF16)
            nc.vector.tensor_mul(bm, b2[:, :, 0:C], mask_all)
            nc.vector.tensor_reduce(bt, bm, axis=mybir.AxisListType.X, op=ALU.add)
            nc.vector.tensor_reduce(SE, a_sb, axis=mybir.AxisListType.X, op=ALU.add)
            nc.vector.tensor_reduce(XS, x_sb, axis=mybir.AxisListType.X, op=ALU.add)

            # ---- phase S: per-batch pair work (sigma table on ACT)
            for b in range(NB):
                lhsT = xt_sb[:, b * P : (b + 1) * P]

                for c in range(NCH_ACT):
                    pt = psum_pool.tile([P, CHUNK], F32, tag="pt")
                    for m in range(CHUNK // MM_N):
                        f0 = c * CHUNK + m * MM_N
                        nc.tensor.matmul(
                            pt[:, m * MM_N : (m + 1) * MM_N],
                            lhsT,
                            w_sb[:, f0 : f0 + MM_N],
                        )
                    sg = work.tile([P, CHUNK], BF16, tag="sg")
                    nc.scalar.activation(sg, pt, AF.Sigmoid)
                    # product tree to groups of 8 (strided groups; range
                    # bound: sigma >= e^-8, 8 factors >= e^-64 > bf16 min)
                    nc.vector.tensor_mul(sg[:, :1024], sg[:, :1024], sg[:, 1024:])
                    nc.vector.tensor_mul(sg[:, :512], sg[:, :512], sg[:, 512:1024])
                    nc.vector.tensor_mul(
                        gs_all[:, b, c * 256 : (c + 1) * 256],
                        sg[:, :256], sg[:, 256:512],
                    )

                # rank-1 chunk: delta 49..64
                u = work.tile([P, CHUNK], BF16, tag="u")
                u_ap = u[:]
                u3 = AP(u_ap.tensor, u_ap.offset, [u_ap.ap[0], [C, 16], [1, C]])
                ab = a_sb[:, b, :].unsqueeze(1).broadcast_to([P, 16, C])
                bb = b2[:, b, :]
                bwin = AP(bb.tensor, bb.offset + 49, [bb.ap[0], [1, 16], [1, C]])
                nc.vector.tensor_mul(u3, ab, bwin)
                # w = (1+u)*e^-LAM, one fused 4x op
                nc.vector.tensor_scalar(u, u, ELAM, ELAM, op0=ALU.mult, op1=ALU.add)
                # delta 49..63 (cols 0:1920), groups of 16
                nc.vector.tensor_mul(u[:, :960], u[:, :960], u[:, 960:1920])
                nc.vector.tensor_mul(u[:, :480], u[:, :480], u[:, 480:960])
                nc.vector.tensor_mul(u[:, :240], u[:, :240], u[:, 240:480])
                nc.vector.tensor_mul(
                    r1_all[:, b, :], u[:, :120], u[:, 120:240]
                )
                # delta 64 (cols 1920:2048), groups of 16, weight 1
                nc.vector.tensor_mul(
                    u[:, 1920:1984], u[:, 1920:1984], u[:, 1984:2048]
                )
                nc.vector.tensor_mul(
                    u[:, 1920:1952], u[:, 1920:1952], u[:, 1952:1984]
                )
                nc.vector.tensor_mul(
                    u[:, 1920:1936], u[:, 1920:1936], u[:, 1936:1952]
                )
                nc.vector.tensor_mul(
                    r64_all[:, b, :], u[:, 1920:1928], u[:, 1928:1936]
                )
                # P1: v = a*b_t + 1, groups of 8
                v = work.tile([P, C], BF16, tag="v")
                nc.vector.tensor_scalar(
                    v, a_sb[:, b, :], bt[:, b : b + 1], 1.0,
                    op0=ALU.mult, op1=ALU.add,
                )
                nc.vector.tensor_mul(v[:, :64], v[:, :64], v[:, 64:128])
                nc.vector.tensor_mul(v[:, :32], v[:, :32], v[:, 32:64])
                nc.vector.tensor_mul(p1_all[:, b, :], v[:, :16], v[:, 16:32])

            # ---- phase L: big Lns with accumulate (ln table on ACT)
            nc.scalar.activation(
                gs_all[:, :, :], gs_all[:, :, :], AF.Ln, accum_out=ACCS[:, 3:4]
            )
            nc.scalar.activation(
                r1_all[:, :, :], r1_all[:, :, :], AF.Ln, accum_out=ACCS[:, 2:3]
            )
            nc.scalar.activation(
                r64_all[:, :, :], r64_all[:, :, :], AF.Ln, accum_out=ACCS[:, 4:5]
            )
            nc.scalar.activation(
                p1_all[:, :, :], p1_all[:, :, :], AF.Ln, accum_out=ACCS[:, 1:2]
            )
            nc.scalar.activation(SE, SE, AF.Ln, accum_out=ACCS[:, 0:1])
            nc.vector.tensor_reduce(
                ACCS[:, 5:6], XS, axis=mybir.AxisListType.X, op=ALU.add
            )
            nc.vector.tensor_reduce(
                ACCS[:, 6:7], Y, axis=mybir.AxisListType.X, op=ALU.add
            )
            nc.vector.memset(ACCS[:, 7:8], 1.0)
            nc.vector.tensor_mul(ACCS, ACCS, cf_sb)
            nc.vector.tensor_reduce(
                Lfin, ACCS, axis=mybir.AxisListType.X, op=ALU.add
            )
            nc.sync.dma_start(out=out_d[:], in_=Lfin)

    nc.compile()
    return nc


def _host_constants():
    if "w" not in _cache:
        w = np.zeros((C, WCOLS), np.float32)
        for d in range(1, NCH_ACT * 16 + 1):
            base = (d - 1) * C
            j = np.arange(C)
            # e = x_{(j+d)%C} - x_j  ->  sigma(e) = sigma(-d_pair)
            w[(j + d) % C, base + j] += 1.0
            w[j, base + j] -= 1.0
        _cache["w"] = w.astype(ml_dtypes.bfloat16)
        _cache["io"] = np.broadcast_to(
            np.arange(C, dtype=np.float32), (P, C)
        ).astype(ml_dtypes.bfloat16).copy()
        coef = np.array(
            [K_CE, K_P1, K_R1, K_S, K_R64, K_SX, K_Y, CONST_L], np.float32
        )
        _cache["cf"] = np.broadcast_to(coef, (P, 8)).copy()
    return _cache["w"], _cache["io"], _cache["cf"]


def kernel(inputs: np.ndarray, targets: np.ndarray) -> np.ndarray:
    x = np.ascontiguousarray(np.asarray(inputs, dtype=np.float32))
    t = np.asarray(targets)
    assert x.shape == (N, C) and t.shape == (N,)

    if "nc" not in _cache:
        _cache["nc"] = _build_program()
    nc = _cache["nc"]
    w, io, cf = _host_constants()

    xt = np.ascontiguousarray(x.T).astype(ml_dtypes.bfloat16)
    tf = t.astype(np.float32)

    in_maps = []
    for c in range(NCORES):
        r0, r1 = c * ROWS, (c + 1) * ROWS
        in_maps.append(
            {
                "x": np.ascontiguousarray(x[r0:r1]),
                "xt": np.ascontiguousarray(xt[:, r0:r1]),
                "w": w,
                "io": io,
                "cf": cf,
                "tf": np.ascontiguousarray(tf[r0:r1]),
            }
        )

    res = run_bass_kernel_spmd(nc, in_maps, list(range(NCORES)))
    total = 0.0
    for c in range(NCORES):
        total += np.sum(res.results[c]["out"].astype(np.float64))
    return np.float32(total / N)


# revision 12
# speedup vs baseline: 1.3757x; 1.1642x over previous
"""CPC loss kernel for Trainium2, data-parallel over 8 NeuronCores.

Math (per row x of shape [C], target t, y = x[t], C = 128, sp(d) = ln(1+e^d)):
  ce   = ln(sum_j e^{x_j}) - y
  bdc  = (P1 - ln2)/(C-1),                P1 = sum_j sp(x_j - y)
  bec  = 0.5*(SP - 2*P1 + S - C*y + ln2)/((C-1)(C-2)),
         SP = sum_{j,k in CxC} sp(x_j - x_k),  S = sum_j x_j

Pair enumeration is CIRCULANT: ordered pairs (j, (j+delta)%C), delta=1..127.
Pairing delta with C-delta and using sp(d)+sp(-d) = 2*ln(1+e^d) - d (the d's
telescope to zero over a full cyclic shift):

  SP = C*ln2 + 2*sum_{delta=1..63} sum_j sp(d_{j,delta})
            + sum_j sp(d_{j,64}),      d_{j,delta} = x_j - x_{(j+delta)%C}

No linear correction terms.  Only the SUM over rows is needed (scalar
output), so per-row quantities accumulate linearly: group products are
buffered across all 16 row-batches and a few big Ln(accum_out) instructions
run once at the end.

Work split per 128-row batch (8192 pair columns = 64 delta-blocks of 128):
  - delta 1..NSB: TensorE matmul W gives e = x_{j+d} - x_j in PSUM; ScalarE
    computes sigma(e) = 1/(1+e^{d}) [one Sigmoid pass, no "+1"]; product
    trees to groups of 8 run on DVE with level 1 optionally on GPSIMD;
    ln sigma = -sp(d).
  - delta NSB+1..64: "rank-1" path with NO ScalarE work per pair:
    u = e^{x_j}*e^{-x_{j+d}} from precomputed a = e^x, b2 = [e^-x, e^-x]
    via a broadcast AP times a sliding-window AP (one DVE mul), then
    w = (1+u)*e^-LAM in one fused 4x tensor_scalar; groups of 8.
    delta=64 (weight 1, not 2) gets its own sub-tree and accumulator.
  - Group-of-8 products centered by e^-LAM stay inside the ACT Ln table's
    ~+-44.4 domain (beyond it the table clamps low / corrupts high).
  - P1 via the same rank-1 trick with b_t = e^{-y-LAM} per-partition scalar.
  - CE: ln(sum_j a_j) - y.
Tables: exp+ln in one set (natural_log_exp_and_others via the chooser
patch), Sigmoid in sigmoid_and_others -> exactly 3 table loads.
Output: per-partition partial sums [P,1]; host sums 128*8 values.
"""

import functools

import numpy as np
import ml_dtypes

import concourse.bass as bass
import concourse.tile as tile
import concourse.hw_specs as hw_specs
from concourse import bacc, mybir
from concourse.ap import AP
from concourse.bass_utils import run_bass_kernel_spmd

_orig_get_activation_tables = hw_specs.get_activation_tables


@functools.cache
def _patched_activation_tables(module_arch: str):
    d = dict(_orig_get_activation_tables(module_arch))
    for name in ("exp_and_others", "natural_log", "exp_and_friends"):
        if name in d:
            d[name] = set()
    return d


hw_specs.get_activation_tables = _patched_activation_tables
bacc.get_activation_tables = _patched_activation_tables

N, C = 16384, 128
NCORES = 8
ROWS = N // NCORES            # rows per core
P = 128                       # partitions / rows per batch
NB = ROWS // P                # batches per core
MM_N = 512                    # moving free dim per matmul (1 PSUM bank)

F32 = mybir.dt.float32
BF16 = mybir.dt.bfloat16
AF = mybir.ActivationFunctionType
ALU = mybir.AluOpType

# ---- tunables (scanned via TimelineSim) ----
NSB = 48                      # sigma-path delta blocks (delta 1..NSB)
SIGMA_L1 = ("split", "split", "split")   # per-chunk level-1 placement
R1_L1 = "dve"                 # rank-1 weight-2 segment level-1 placement

LAM = 4.4
ELAM = float(np.exp(-LAM))
LOG2 = float(np.log(2.0))
M2 = (C - 1) * (C - 2)

_cache: dict = {}


def _derived():
    RB = 64 - NSB                      # rank-1 delta blocks
    scols = NSB * C                    # sigma columns
    sizes = []
    left = scols
    while left > 0:
        sizes.append(min(2048, left))
        left -= 2048
    rcols = RB * C                     # rank-1 columns
    seg2 = (RB - 1) * C                # weight-2 segment
    return RB, scols, sizes, rcols, seg2


def _consts():
    RB, scols, sizes, rcols, seg2 = _derived()
    K1 = NB * seg2 * LAM
    K64 = NB * C * LAM
    KP1C = NB * C * LAM
    CONST_T = 2.0 * K1 + K64 + NB * C * LOG2 + NB * LOG2
    K_CE = 1.0
    K_P1 = 1.0 / (C - 1) - 1.0 / M2
    K_R1 = 1.0 / M2
    K_S = -1.0 / M2
    K_R64 = 0.5 / M2
    K_SX = 0.5 / M2
    K_Y = -1.0 - 0.5 * C / M2
    CONST_L = -NB * LOG2 / (C - 1) + 0.5 * CONST_T / M2 + K_P1 * KP1C
    return [K_CE, K_P1, K_R1, K_S, K_R64, K_SX, K_Y, CONST_L]


def _build_program() -> bass.Bass:
    RB, scols, sizes, rcols, seg2 = _derived()
    nc = bacc.Bacc("TRN2")

    x_d = nc.declare_dram_parameter("x", [ROWS, C], F32, isOutput=False)
    xt_d = nc.declare_dram_parameter("xt", [C, ROWS], BF16, isOutput=False)
    w_d = nc.declare_dram_parameter("w", [C, scols], BF16, isOutput=False)
    io_d = nc.declare_dram_parameter("io", [P, C], BF16, isOutput=False)
    cf_d = nc.declare_dram_parameter("cf", [P, 8], F32, isOutput=False)
    tf_d = nc.declare_dram_parameter("tf", [ROWS], F32, isOutput=False)
    out_d = nc.declare_dram_parameter("out", [P, 1], F32, isOutput=True)

    with tile.TileContext(nc) as tc:
        with (
            tc.tile_pool(name="const", bufs=1) as const_pool,
            tc.tile_pool(name="work", bufs=3) as work,
            tc.tile_pool(name="acc", bufs=1) as acc_pool,
            tc.tile_pool(name="psum", bufs=2, space="PSUM") as psum_pool,
        ):
            # x first: the exp phase depends only on it
            x_sb = const_pool.tile([P, NB, C], F32)
            nc.sync.dma_start(out=x_sb, in_=x_d.rearrange("(b p) c -> p b c", p=P))
            io_sb = const_pool.tile([P, C], BF16)
            nc.sync.dma_start(out=io_sb, in_=io_d[:])
            cf_sb = const_pool.tile([P, 8], F32)
            nc.sync.dma_start(out=cf_sb, in_=cf_d[:])
            t_sb = const_pool.tile([P, NB], F32)
            nc.sync.dma_start(out=t_sb, in_=tf_d.rearrange("(b p) -> p b", p=P))
            xt_sb = const_pool.tile([C, ROWS], BF16)
            nc.sync.dma_start(out=xt_sb, in_=xt_d[:])
            w_sb = const_pool.tile([C, scols], BF16)
            off = 0
            for sz in sizes:
                nc.sync.dma_start(
                    out=w_sb[:, off : off + sz], in_=w_d[:, off : off + sz]
                )
                off += sz

            a_sb = acc_pool.tile([P, NB, C], BF16)       # e^x
            b2 = acc_pool.tile([P, NB, 2 * C], BF16)     # [e^-x, e^-x]
            bt = acc_pool.tile([P, NB], F32)             # e^{-y-LAM} per row
            Y = acc_pool.tile([P, NB], F32)              # y per row
            SE = acc_pool.tile([P, NB], F32)             # sum_j e^x per row
            XS = acc_pool.tile([P, NB], F32)             # sum_j x per row
            gs_all = acc_pool.tile([P, NB, scols // 8], BF16)
            r1_all = acc_pool.tile([P, NB, seg2 // 8], BF16)
            r64_all = acc_pool.tile([P, NB, 16], BF16)
            p1_all = acc_pool.tile([P, NB, 16], BF16)
            ACCS = acc_pool.tile([P, 8], F32)            # CE,P1,R1,S,R64,SX,Y,1
            mask_all = acc_pool.tile([P, NB, C], BF16)
            Lfin = acc_pool.tile([P, 1], F32)

            # ---- phase E: just the exps (exp table on ACT) so batch 0's
            # matmul/sigma pipeline starts immediately
            nc.scalar.activation(a_sb[:, :, :], x_sb[:, :, :], AF.Exp)
            nc.scalar.activation(
                b2[:, :, 0:C], x_sb[:, :, :], AF.Exp, bias=0.0, scale=-1.0
            )
            nc.vector.tensor_copy(b2[:, :, C : 2 * C], b2[:, :, 0:C])

            # ---- phase S: per-batch pair work (sigma table on ACT)
            for b in range(NB):
                lhsT = xt_sb[:, b * P : (b + 1) * P]

                goff = 0
                for ci, sz in enumerate(sizes):
                    pt = psum_pool.tile([P, 2048], F32, tag="pt")
                    for m in range(sz // MM_N):
                        f0 = sum(sizes[:ci]) + m * MM_N
                        nc.tensor.matmul(
                            pt[:, m * MM_N : (m + 1) * MM_N],
                            lhsT,
                            w_sb[:, f0 : f0 + MM_N],
                        )
                    sg = work.tile([P, 2048], BF16, tag="sg")
                    nc.scalar.activation(sg[:, :sz], pt[:, :sz], AF.Sigmoid)
                    h, q, g = sz // 2, sz // 4, sz // 8
                    plc = SIGMA_L1[ci]
                    if plc == "pool":
                        nc.gpsimd.tensor_mul(sg[:, :h], sg[:, :h], sg[:, h:sz])
                    elif plc == "dve":
                        nc.vector.tensor_mul(sg[:, :h], sg[:, :h], sg[:, h:sz])
                    else:  # split: pool low half, dve high half
                        hh = h // 2
                        nc.gpsimd.tensor_mul(
                            sg[:, :hh], sg[:, :hh], sg[:, h : h + hh]
                        )
                        nc.vector.tensor_mul(
                            sg[:, hh:h], sg[:, hh:h], sg[:, h + hh : sz]
                        )
                    nc.vector.tensor_mul(sg[:, :q], sg[:, :q], sg[:, q:h])
                    nc.vector.tensor_mul(
                        gs_all[:, b, goff : goff + g], sg[:, :q // 2], sg[:, q // 2 : q]
                    )
                    goff += g

                # rank-1 chunk: delta NSB+1..64
                u = work.tile([P, rcols], BF16, tag="u")
                u_ap = u[:]
                u3 = AP(u_ap.tensor, u_ap.offset, [u_ap.ap[0], [C, RB], [1, C]])
                ab = a_sb[:, b, :].unsqueeze(1).broadcast_to([P, RB, C])
                bb = b2[:, b, :]
                bwin = AP(
                    bb.tensor, bb.offset + NSB + 1, [bb.ap[0], [1, RB], [1, C]]
                )
                nc.vector.tensor_mul(u3, ab, bwin)
                # w = (1+u)*e^-LAM, one fused 4x op
                nc.vector.tensor_scalar(u, u, ELAM, ELAM, op0=ALU.mult, op1=ALU.add)
                # delta NSB+1..63 (cols 0:seg2), groups of 8
                s2, s4, s8 = seg2 // 2, seg2 // 4, seg2 // 8
                if R1_L1 == "pool":
                    nc.gpsimd.tensor_mul(u[:, :s2], u[:, :s2], u[:, s2:seg2])
                else:
                    nc.vector.tensor_mul(u[:, :s2], u[:, :s2], u[:, s2:seg2])
                nc.vector.tensor_mul(u[:, :s4], u[:, :s4], u[:, s4:s2])
                nc.vector.tensor_mul(r1_all[:, b, :], u[:, :s8], u[:, s8:s4])
                # delta 64 (last 128 cols), groups of 8, weight 1
                e0 = seg2
                nc.vector.tensor_mul(
                    u[:, e0 : e0 + 64], u[:, e0 : e0 + 64], u[:, e0 + 64 : e0 + 128]
                )
                nc.vector.tensor_mul(
                    u[:, e0 : e0 + 32], u[:, e0 : e0 + 32], u[:, e0 + 32 : e0 + 64]
                )
                nc.vector.tensor_mul(
                    r64_all[:, b, :], u[:, e0 : e0 + 16], u[:, e0 + 16 : e0 + 32]
                )

            # ---- post-loop gathers + P1 (DVE) — these overlap the ACT-only
            # Ln tail below, instead of delaying batch 0 at the start
            for b in range(NB):
                nc.vector.tensor_scalar(
                    mask_all[:, b, :], io_sb, t_sb[:, b : b + 1], None,
                    op0=ALU.is_equal,
                )
            xm = acc_pool.tile([P, NB, C], F32)
            nc.vector.tensor_mul(xm, x_sb, mask_all)
            nc.vector.tensor_reduce(Y, xm, axis=mybir.AxisListType.X, op=ALU.add)
            bm = acc_pool.tile([P, NB, C], BF16)
            nc.vector.tensor_mul(bm, b2[:, :, 0:C], mask_all)
            nc.vector.tensor_reduce(bt, bm, axis=mybir.AxisListType.X, op=ALU.add)
            # bts = e^-y * e^-LAM so P1's v = a*bts + e^-LAM is centered too
            nc.vector.tensor_scalar_mul(bt, bt, ELAM)
            nc.vector.tensor_reduce(SE, a_sb, axis=mybir.AxisListType.X, op=ALU.add)
            nc.vector.tensor_reduce(XS, x_sb, axis=mybir.AxisListType.X, op=ALU.add)
            # P1: v = (a*e^-y + 1)*e^-LAM, groups of 8
            for b in range(NB):
                v = work.tile([P, C], BF16, tag="v")
                nc.vector.tensor_scalar(
                    v, a_sb[:, b, :], bt[:, b : b + 1], ELAM,
                    op0=ALU.mult, op1=ALU.add,
                )
                nc.vector.tensor_mul(v[:, :64], v[:, :64], v[:, 64:128])
                nc.vector.tensor_mul(v[:, :32], v[:, :32], v[:, 32:64])
                nc.vector.tensor_mul(p1_all[:, b, :], v[:, :16], v[:, 16:32])

            # ---- phase L: big Lns with accumulate (ln table on ACT)
            nc.scalar.activation(
                gs_all[:, :, :], gs_all[:, :, :], AF.Ln, accum_out=ACCS[:, 3:4]
            )
            nc.scalar.activation(
                r1_all[:, :, :], r1_all[:, :, :], AF.Ln, accum_out=ACCS[:, 2:3]
            )
            nc.scalar.activation(
                r64_all[:, :, :], r64_all[:, :, :], AF.Ln, accum_out=ACCS[:, 4:5]
            )
            nc.scalar.activation(
                p1_all[:, :, :], p1_all[:, :, :], AF.Ln, accum_out=ACCS[:, 1:2]
            )
            nc.scalar.activation(SE, SE, AF.Ln, accum_out=ACCS[:, 0:1])
            nc.vector.tensor_reduce(
                ACCS[:, 5:6], XS, axis=mybir.AxisListType.X, op=ALU.add
            )
            nc.vector.tensor_reduce(
                ACCS[:, 6:7], Y, axis=mybir.AxisListType.X, op=ALU.add
            )
            nc.vector.memset(ACCS[:, 7:8], 1.0)
            nc.vector.tensor_mul(ACCS, ACCS, cf_sb)
            nc.vector.tensor_reduce(
                Lfin, ACCS, axis=mybir.AxisListType.X, op=ALU.add
            )
            nc.sync.dma_start(out=out_d[:], in_=Lfin)

    nc.compile()
    return nc


def _host_constants():
    RB, scols, sizes, rcols, seg2 = _derived()
    if _cache.get("w_nsb") != NSB:
        w = np.zeros((C, scols), np.float32)
        for d in range(1, NSB + 1):
            base = (d - 1) * C
            j = np.arange(C)
            # e = x_{(j+d)%C} - x_j  ->  sigma(e) = sigma(-d_pair)
            w[(j + d) % C, base + j] += 1.0
            w[j, base + j] -= 1.0
        _cache["w"] = w.astype(ml_dtypes.bfloat16)
        _cache["io"] = np.broadcast_to(
            np.arange(C, dtype=np.float32), (P, C)
        ).astype(ml_dtypes.bfloat16).copy()
        _cache["cf"] = np.broadcast_to(
            np.array(_consts(), np.float32), (P, 8)
        ).copy()
        _cache["w_nsb"] = NSB
    return _cache["w"], _cache["io"], _cache["cf"]


def kernel(inputs: np.ndarray, targets: np.ndarray) -> np.ndarray:
    x = np.ascontiguousarray(np.asarray(inputs, dtype=np.float32))
    t = np.asarray(targets)
    assert x.shape == (N, C) and t.shape == (N,)

    if "nc" not in _cache:
        _cache["nc"] = _build_program()
    nc = _cache["nc"]
    w, io, cf = _host_constants()

    xt = np.ascontiguousarray(x.T).astype(ml_dtypes.bfloat16)
    tf = t.astype(np.float32)

    in_maps = []
    for c in range(NCORES):
        r0, r1 = c * ROWS, (c + 1) * ROWS
        in_maps.append(
            {
                "x": np.ascontiguousarray(x[r0:r1]),
                "xt": np.ascontiguousarray(xt[:, r0:r1]),
                "w": w,
                "io": io,
                "cf": cf,
                "tf": np.ascontiguousarray(tf[r0:r1]),
            }
        )

    res = run_bass_kernel_spmd(nc, in_maps, list(range(NCORES)))
    total = 0.0
    for c in range(NCORES):
        total += np.sum(res.results[c]["out"].astype(np.float64))
    return np.float32(total / N)
